# revision 1
# baseline (speedup 1.0000x reference)
"""MoE routed-expert kernel for Trainium2 (8 NeuronCores, SPMD).

Problem: N=16384 tokens, D=768, H=768, C=2, E=20 experts.
  y[n] = relu(x[n] @ W1[e] + b1[e]) @ W2[e] + b2[e],  e = component_idx[n]

Strategy
--------
Host side (numpy): sort tokens by expert, split the token groups into
8*n_slots fragments (splitting the largest until the count matches), deal
the size-sorted fragments into 8 cores x n_slots "expert slots" with a
uniform per-slot capacity = max fragment size in that slot. Every core then
runs the SAME static program (SPMD); which expert a slot holds is purely a
matter of which weights/tokens the host stages into that core's input
buffers. Padding waste is ~2%.

Device side (Bass/Tile, per core): for each slot, load that expert's
W1 [768,768] (+ b1, W2, b2) and the slot's token block x^T with D on
partitions and tokens on the free dim. For each chunk of 256..512 tokens:
  layer1: 6x6 accumulating fp16 matmuls (full PE speed, FWL weight loads)
  relu+bias fused on ScalarE (PSUM fp32 -> SBUF fp16)
  layer2: 6 accumulating matmuls into a [2, T] PSUM, bias via VectorE
One DMA writes the [2, R] result strip back; the host scatters rows to the
original token order.
"""

import math

import numpy as np

import concourse.bass as bass  # noqa: F401
import concourse.mybir as mybir
from concourse import bacc
from concourse.bass_utils import run_bass_kernel_spmd
from concourse.tile import TileContext

F32 = mybir.dt.float32
F32R = mybir.dt.float32r
F16 = mybir.dt.float16
MM_DT = F16
MM_NP = np.float16

N_CORES = 8
N_SLOTS = 3
D = 768
H = 768
C = 2
DT = D // 128  # 6 d-tiles
HT = H // 128  # 6 h-tiles
MAX_CHUNK = 512  # one PSUM bank holds 512 fp32 -> matmul free dim cap
MIN_CAP = 256  # keep chunks >=256 so per-matmul overhead stays amortized

# Layer-2 via 4 concurrent PE column groups (tile_position) + selector-matmul
# reduction: 3T cycles per chunk instead of 6T.
L2_PACK = False
L2_M = 32  # pad W2's C=2 cols to a full 32-wide column group

# Issue slot-0 payload DMAs from GpSimd: its sequencer starts ~3us before
# Sync's in the launch preamble, pulling the fill forward.
EARLY_DMA = False
WARMUP = True


def _round_cap(cap: int) -> int:
    """Round capacity up so it splits into equal, even chunks of 256..512
    (float32r matmul needs an even moving free dim, and >=256 for speed)."""
    cap = max(cap, MIN_CAP)
    n = max(1, math.ceil(cap / MAX_CHUNK))
    return 2 * n * math.ceil(cap / (2 * n))


def _chunk_sizes(cap: int, tail_split: bool = False) -> list[int]:
    n = max(1, math.ceil(cap / MAX_CHUNK))
    assert cap % n == 0 and (cap // n) % 2 == 0, cap
    sizes = [cap // n] * n
    if tail_split and sizes[-1] > 256:
        # shorten the serial relu->L2->add->DMA chain after the last
        # layer-1 matmul of the program (fp16 allows any free dim)
        sizes[-1:] = [sizes[-1] - 128, 128]
    return sizes


def _plan_packing(counts: np.ndarray):
    """Return (caps, assign): per-slot capacities and
    assign[s][c] = (expert, start_within_group, length)."""
    frags = [(int(e), 0, int(c)) for e, c in enumerate(counts) if c > 0]
    target = N_CORES * N_SLOTS
    assert len(frags) <= target, (
        f"{len(frags)} non-empty experts exceed {target} slots; raise N_SLOTS"
    )
    while len(frags) < target:
        frags.sort(key=lambda f: -f[2])
        e, st, ln = frags[0]
        if ln < 2:
            frags.append((e, st, 0))
            continue
        h1 = ln // 2
        frags[0] = (e, st, ln - h1)
        frags.append((e, st + (ln - h1), h1))
    frags.sort(key=lambda f: -f[2])
    caps, assign = [], []
    for s in range(N_SLOTS):
        group = frags[s * N_CORES : (s + 1) * N_CORES]
        caps.append(_round_cap(max(f[2] for f in group)))
        assign.append(group)
    return caps, assign


_PROGRAM_CACHE: dict = {}


def _build_program(caps: tuple):
    if caps in _PROGRAM_CACHE:
        return _PROGRAM_CACHE[caps]

    R = sum(caps)
    nc = bacc.Bacc(
        "TRN2", target_bir_lowering=False, debug=False, num_devices=N_CORES
    )
    xT = nc.dram_tensor("xT", [DT, 128, R], MM_DT, kind="ExternalInput")
    w1 = nc.dram_tensor("w1", [N_SLOTS, DT, 128, H], MM_DT, kind="ExternalInput")
    b1 = nc.dram_tensor("b1", [N_SLOTS, 128, HT], F32, kind="ExternalInput")
    w2_m = L2_M if L2_PACK else C
    w2 = nc.dram_tensor("w2", [N_SLOTS, 128, HT, w2_m], MM_DT, kind="ExternalInput")
    if L2_PACK:
        sel = nc.dram_tensor("sel", [128, C], MM_DT, kind="ExternalInput")
    b2 = nc.dram_tensor("b2", [N_SLOTS, C, 1], F32, kind="ExternalInput")
    y = nc.dram_tensor("y", [C, R], F32, kind="ExternalOutput")

    with TileContext(nc) as tc:
        with (
            tc.tile_pool(name="wpool", bufs=2) as wpool,
            tc.tile_pool(name="xpool", bufs=2) as xpool,
            tc.tile_pool(name="hpool", bufs=4) as hpool,
            tc.tile_pool(name="ypool", bufs=1) as ypool,
            tc.tile_pool(name="pspool", bufs=7, space="PSUM") as pspool,
            tc.tile_pool(name="pypool", bufs=1, space="PSUM") as pypool,
        ):
            y_sb = ypool.tile([C, R], F32, name="y_sb")
            if L2_PACK:
                sel_sb = ypool.tile([128, C], MM_DT, name="sel_sb")
                nc.sync.dma_start(out=sel_sb[:, :], in_=sel[:, :])

            if WARMUP:
                # PE warm-up during the launch preamble: dummy matmuls flip
                # the HAM clock-gate to 2.4 GHz before real data lands.
                wu_w = ypool.tile([128, 128], F32, name="wu_w")
                wu_x = ypool.tile([128, 256], F32, name="wu_x")
                nc.gpsimd.memset(wu_w[:, :], 0.0)
                nc.gpsimd.memset(wu_x[:, :], 0.0)
                wu_ps = pypool.tile([128, 256], F32, name="wu_ps", tag="psy")
                for _ in range(4):
                    nc.tensor.matmul(wu_ps, wu_w, wu_x, start=True, stop=True)

            off = 0
            for s in range(N_SLOTS):
                cap = caps[s]
                # per-d-tile slabs: precise deps so the first matmuls can
                # start as soon as the first ~0.8MB lands
                w1_d, xs_d = [], []
                for dt in range(DT):
                    w1t = wpool.tile(
                        [128, H], MM_DT, name=f"w1_d{dt}", tag=f"w1d{dt}"
                    )
                    nc.sync.dma_start(out=w1t, in_=w1[s, dt])
                    xst = xpool.tile(
                        [128, cap], MM_DT, name=f"xs_d{dt}", tag=f"xsd{dt}"
                    )
                    nc.sync.dma_start(out=xst, in_=xT[dt, :, off : off + cap])
                    w1_d.append(w1t)
                    xs_d.append(xst)
                b1_sb = wpool.tile([128, HT], F32, name="b1_sb", tag="b1")
                nc.sync.dma_start(out=b1_sb, in_=b1[s])
                w2_sb = wpool.tile([128, HT, w2_m], MM_DT, name="w2_sb", tag="w2")
                nc.sync.dma_start(out=w2_sb, in_=w2[s])
                b2_sb = wpool.tile([C, 1], F32, name="b2_sb", tag="b2")
                nc.sync.dma_start(out=b2_sb, in_=b2[s])

                co = 0
                for ci, size in enumerate(
                    _chunk_sizes(cap, tail_split=(s == N_SLOTS - 1))
                ):
                    h_sb = hpool.tile([128, HT, size], MM_DT, name="h_sb", tag="h")
                    if s == 0 and ci == 0:
                        # dt-major: each dt round needs only that dt's two
                        # slabs -> PE starts while later slabs still stream
                        ps_list = [
                            pspool.tile(
                                [128, size], F32, name=f"ps_h{ht}", tag="psh"
                            )
                            for ht in range(HT)
                        ]
                        for dt in range(DT):
                            for ht in range(HT):
                                nc.tensor.matmul(
                                    ps_list[ht],
                                    w1_d[dt][:, ht * 128 : (ht + 1) * 128],
                                    xs_d[dt][:, co : co + size],
                                    start=(dt == 0),
                                    stop=(dt == DT - 1),
                                )
                        for ht in range(HT):
                            nc.scalar.activation(
                                h_sb[:, ht, :],
                                ps_list[ht],
                                mybir.ActivationFunctionType.Relu,
                                bias=b1_sb[:, ht : ht + 1],
                            )
                    else:
                        for ht in range(HT):
                            ps_h = pspool.tile(
                                [128, size], F32, name="ps_h", tag="psh"
                            )
                            for dt in range(DT):
                                nc.tensor.matmul(
                                    ps_h,
                                    w1_d[dt][:, ht * 128 : (ht + 1) * 128],
                                    xs_d[dt][:, co : co + size],
                                    start=(dt == 0),
                                    stop=(dt == DT - 1),
                                )
                            nc.scalar.activation(
                                h_sb[:, ht, :],
                                ps_h,
                                mybir.ActivationFunctionType.Relu,
                                bias=b1_sb[:, ht : ht + 1],
                            )
                    if L2_PACK:
                        # 4 concurrent column-group matmuls (2 rounds), then
                        # a selector matmul folds the 4 partials into [C, T]
                        ps_y4 = pypool.tile(
                            [128, size], F32, name="ps_y4", tag="psy"
                        )
                        for ht in range(HT):
                            g = ht % 4
                            nc.tensor.matmul(
                                ps_y4[32 * g : 32 * g + L2_M, :],
                                w2_sb[:, ht, :],
                                h_sb[:, ht, :],
                                start=(ht < 4),
                                stop=(ht >= 4 or g >= HT - 4),
                                tile_position=(0, 32 * g),
                            )
                        y4_sb = hpool.tile(
                            [128, size], MM_DT, name="y4_sb", tag="y4"
                        )
                        nc.scalar.activation(
                            y4_sb, ps_y4, mybir.ActivationFunctionType.Copy
                        )
                        ps_y = pypool.tile(
                            [C, size], F32, name="ps_y", tag="psy2"
                        )
                        nc.tensor.matmul(
                            ps_y, sel_sb, y4_sb, start=True, stop=True
                        )
                    else:
                        ps_y = pypool.tile(
                            [C, size], F32, name="ps_y", tag="psy"
                        )
                        for ht in range(HT):
                            nc.tensor.matmul(
                                ps_y,
                                w2_sb[:, ht, :],
                                h_sb[:, ht, :],
                                start=(ht == 0),
                                stop=(ht == HT - 1),
                            )
                    nc.vector.tensor_scalar_add(
                        y_sb[:, off + co : off + co + size], ps_y, b2_sb[:, :]
                    )
                    if s == N_SLOTS - 1:
                        # last slot: write back per chunk so only the final
                        # ~1KB DMA sits on the exit chain
                        nc.sync.dma_start(
                            out=y[:, off + co : off + co + size],
                            in_=y_sb[:, off + co : off + co + size],
                        )
                    co += size
                if s != N_SLOTS - 1:
                    nc.sync.dma_start(
                        out=y[:, off : off + cap], in_=y_sb[:, off : off + cap]
                    )
                off += cap
    nc.compile()
    _PROGRAM_CACHE[caps] = nc
    return nc


def kernel(embeddings, component_idx, W1, b1, W2, b2):
    embeddings = np.ascontiguousarray(np.asarray(embeddings, dtype=np.float32))
    ci = np.asarray(component_idx).astype(np.int64, copy=False)
    W1 = np.asarray(W1, dtype=np.float32)
    b1 = np.asarray(b1, dtype=np.float32)
    W2 = np.asarray(W2, dtype=np.float32)
    b2 = np.asarray(b2, dtype=np.float32)

    N = embeddings.shape[0]
    E = W1.shape[0]

    counts = np.bincount(ci, minlength=E)
    order = np.argsort(ci, kind="stable")
    group_start = np.zeros(E, dtype=np.int64)
    group_start[1:] = np.cumsum(counts)[:-1]
    x_sorted = embeddings[order]  # [N, D] grouped by expert

    caps, assign = _plan_packing(counts)
    R = sum(caps)
    offs = np.cumsum([0] + caps[:-1]).tolist() if len(caps) > 1 else [0]

    nc = _build_program(tuple(caps))

    # host-side packing of per-core inputs
    w1_packed = W1.reshape(E, DT, 128, H).astype(MM_NP)  # [e, dt, din, h]
    b1_packed = np.ascontiguousarray(
        b1.reshape(E, HT, 128).transpose(0, 2, 1)
    )  # [e, 128, ht]
    w2_m = L2_M if L2_PACK else C
    w2_packed = np.zeros((E, 128, HT, w2_m), dtype=MM_NP)
    w2_packed[:, :, :, :C] = W2.reshape(E, HT, 128, C).transpose(0, 2, 1, 3)
    b2_packed = b2.reshape(E, C, 1)

    in_maps = []
    for c in range(N_CORES):
        Xc = np.zeros((R, D), dtype=MM_NP)
        w1_in = np.empty((N_SLOTS, DT, 128, H), dtype=MM_NP)
        b1_in = np.empty((N_SLOTS, 128, HT), dtype=np.float32)
        w2_in = np.empty((N_SLOTS, 128, HT, w2_m), dtype=MM_NP)
        b2_in = np.empty((N_SLOTS, C, 1), dtype=np.float32)
        for s in range(N_SLOTS):
            e, st, ln = assign[s][c]
            beg = group_start[e] + st
            Xc[offs[s] : offs[s] + ln] = x_sorted[beg : beg + ln]
            w1_in[s] = w1_packed[e]
            b1_in[s] = b1_packed[e]
            w2_in[s] = w2_packed[e]
            b2_in[s] = b2_packed[e]
        xT_in = np.ascontiguousarray(Xc.T).reshape(DT, 128, R)
        im = {"xT": xT_in, "w1": w1_in, "b1": b1_in, "w2": w2_in, "b2": b2_in}
        if L2_PACK:
            sel_np = np.zeros((128, C), dtype=MM_NP)
            for g in range(4):
                for cc in range(C):
                    sel_np[32 * g + cc, cc] = 1
            im["sel"] = sel_np
        in_maps.append(im)

    global _LAST_IN_MAPS
    _LAST_IN_MAPS = in_maps
    res = run_bass_kernel_spmd(nc, in_maps, list(range(N_CORES)))

    out = np.empty((N, C), dtype=np.float32)
    for c in range(N_CORES):
        yc = res.results[c]["y"]  # [C, R]
        for s in range(N_SLOTS):
            e, st, ln = assign[s][c]
            beg = group_start[e] + st
            tokens = order[beg : beg + ln]
            out[tokens] = yc[:, offs[s] : offs[s] + ln].T
    return out

